# revision 1
# baseline (speedup 1.0000x reference)
"""Trainium2 Bass kernel for nn_Net_43052752175597 (2-layer GraphSAGE, aggr=add).

reference:
    A_hat = (A != 0).T with unit diagonal          # [N, N], binary
    h1   = X @ W1;  agg1 = A_hat @ h1 + b1;  x1 = relu(l2norm(agg1))
    h2   = x1 @ W2; agg2 = A_hat @ h2 + b2;  out = l2norm(l2norm(agg2))

Sharding: row-shard A_hat (output nodes) across 8 cores. Each core owns a
contiguous block of 1280 padded nodes (N padded 10000 -> 10240). Per layer the
core computes h for its own nodes, AllGathers the transformed features, then
aggregates its A_hat row-slice (streamed as stationary 128x128 tiles) against
the gathered features.

Precision: A is exactly 0/1 so 16-bit A tiles are exact. The dense h1/h2
matmuls run as bf16 hi+lo products (~f32 accurate); the big aggregation
streams h either as a single fp16 pass (AGG_MODE="fp16", h rounding 2^-11)
or as a bf16 hi|lo double pass (AGG_MODE="bf16x2", ~f32), accumulating in
f32 PSUM either way.
"""

import sys

sys.path.insert(0, "/opt/trn_rl_repo")

import numpy as np
import ml_dtypes

import concourse.bass as bass
import concourse.tile as tile
from concourse import bacc, mybir
from concourse import bass_utils

N = 10000
NP = 10240          # padded node count
F = 256             # input feature dim
H = 128             # hidden dim
N_CORES = 8
PER_CORE = NP // N_CORES        # 1280 nodes per core
M_TILES = PER_CORE // 128       # 10
K_TILES = NP // 128             # 80

BF16 = ml_dtypes.bfloat16

# Aggregation precision: "fp16" = single-pass fp16 h (fast, ~3e-4 rel err),
# "bf16x2" = bf16 hi+lo double-pass (~6e-6 rel err, ~30% slower).
AGG_MODE = "fp16"

_CACHE = {}


def _split_hilo(x_f32):
    """Split f32 array into bf16 hi + bf16 lo with x ~= hi + lo."""
    hi = x_f32.astype(BF16)
    lo = (x_f32 - hi.astype(np.float32)).astype(BF16)
    return hi, lo


def _build_nc(agg_mode, single_core=False, compile=True, repeats=1):
    """Build + compile the 8-core SPMD Bass kernel. Returns the Bacc object.

    single_core=True builds a 1-core variant with the collectives replaced by
    equivalent-byte local DMAs — only for TimelineSim cost-model profiling.
    repeats>1 runs the whole 2-layer body N times (benchmarking only).
    """
    fp32 = mybir.dt.float32
    bf16 = mybir.dt.bfloat16
    fp16 = mybir.dt.float16
    fp8 = mybir.dt.float8e4

    hilo = agg_mode == "bf16x2"
    a_dt = bf16 if hilo else fp8     # A tiles: binary, exact in any format
    h_dt = bf16 if hilo else fp16    # streamed h dtype
    HW = 2 * H if hilo else H        # stored width of h rows (hi|lo or single)

    nc = bacc.Bacc(
        "TRN2",
        target_bir_lowering=False,
        debug=False,
        enable_asserts=True,
        num_devices=1 if single_core else N_CORES,
    )

    # Per-core external inputs
    a_pre = nc.dram_tensor("a_pre", [M_TILES, 128, K_TILES, 128], a_dt,
                           kind="ExternalInput").ap()
    xt_hi = nc.dram_tensor("xt_hi", [128, 2, PER_CORE], bf16,
                           kind="ExternalInput").ap()
    xt_lo = nc.dram_tensor("xt_lo", [128, 2, PER_CORE], bf16,
                           kind="ExternalInput").ap()
    w1 = nc.dram_tensor("w1", [128, 2, 2 * H], bf16, kind="ExternalInput").ap()
    w2 = nc.dram_tensor("w2", [128, 2 * H], bf16, kind="ExternalInput").ap()
    b1 = nc.dram_tensor("b1", [128, H], fp32, kind="ExternalInput").ap()
    b2 = nc.dram_tensor("b2", [128, H], fp32, kind="ExternalInput").ap()
    ident = nc.dram_tensor("ident", [128, 128], fp32, kind="ExternalInput").ap()
    out = nc.dram_tensor("out", [PER_CORE, H], fp32, kind="ExternalOutput").ap()

    with tile.TileContext(nc) as tc:
        with tc.tile_pool(name="const", bufs=1) as cpool, \
             tc.tile_pool(name="hfull", bufs=1) as hpool, \
             tc.tile_pool(name="acol", bufs=3) as apool, \
             tc.tile_pool(name="work", bufs=1) as wpool, \
             tc.tile_pool(name="shard", bufs=2) as spool, \
             tc.tile_pool(name="psum_agg", bufs=2, space="PSUM") as pagg, \
             tc.tile_pool(name="psum_aux", bufs=2, space="PSUM") as paux, \
             tc.tile_pool(name="psum_tr", bufs=2, space="PSUM") as ptr, \
             tc.tile_pool(name="dram", bufs=2, space="DRAM") as dpool:

            # ---- constants into SBUF ----
            t_xt_hi = cpool.tile([128, 2, PER_CORE], bf16)
            t_xt_lo = cpool.tile([128, 2, PER_CORE], bf16)
            t_w1 = cpool.tile([128, 2, 2 * H], bf16)
            t_w2 = cpool.tile([128, 2 * H], bf16)
            t_b1 = cpool.tile([128, H], fp32)
            t_b2 = cpool.tile([128, H], fp32)
            t_id = cpool.tile([128, 128], fp32)
            nc.sync.dma_start(t_xt_hi[:], xt_hi[:])
            nc.sync.dma_start(t_xt_lo[:], xt_lo[:])
            nc.sync.dma_start(t_w1[:], w1[:])
            nc.sync.dma_start(t_w2[:], w2[:])
            nc.sync.dma_start(t_b1[:], b1[:])
            nc.sync.dma_start(t_b2[:], b2[:])
            nc.sync.dma_start(t_id[:], ident[:])

            # gathered features, reused for both layers
            t_hfull = hpool.tile([128, K_TILES, HW], h_dt)

            # fp8 mode: whole per-core A slice stays resident in SBUF, loaded
            # once (column-wise so early m-tiles can start ASAP) and reused by
            # both layers.
            t_acache = None
            if not hilo:
                t_acache = apool.tile([128, M_TILES, K_TILES, 128], a_dt,
                                      bufs=1)
                for m in range(M_TILES):
                    # SWDGE so these bulk loads don't queue ahead of the
                    # latency-critical HWDGE gather/feature DMAs.
                    nc.gpsimd.dma_start(t_acache[:, m, :, :], a_pre[m])

            # per-layer working tiles
            t_agg = wpool.tile([128, M_TILES, H], fp32, tag="agg")
            t_x1 = wpool.tile([128, M_TILES, H], fp32)
            t_x1t_hi = wpool.tile([128, M_TILES, H], bf16)
            t_x1t_lo = wpool.tile([128, M_TILES, H], bf16)
            t_ssq = wpool.tile([128, M_TILES], fp32, tag="ssq")
            t_nrm = wpool.tile([128, M_TILES], fp32, tag="nrm")
            t_inv = wpool.tile([128, M_TILES], fp32, tag="inv")
            t_sq_scratch = wpool.tile([128, H], fp32)
            t_outf = wpool.tile([128, M_TILES, H], fp32)

            def compute_h_shard(m, lhs_hi, lhs_lo, rhs_cat, n_k, h_shard):
                """h[:, m] = (lhs_hi+lhs_lo)^T @ W via 3-term hi/lo product.

                lhs_hi/lhs_lo: callables k -> AP [128, 128] (stationary)
                rhs_cat: callable k -> AP [128, 2H] = [W_hi | W_lo]
                Writes the aggregation-dtype encoding into h_shard[:, m, :].
                """
                ps = paux.tile([128, 2 * H], fp32, tag="ps_h")
                for k in range(n_k):
                    nc.tensor.matmul(ps[:], lhs_hi(k), rhs_cat(k),
                                     start=(k == 0), stop=False)
                for k in range(n_k):
                    nc.tensor.matmul(ps[:, 0:H], lhs_lo(k), rhs_cat(k)[:, 0:H],
                                     start=False, stop=(k == n_k - 1))
                hf = wpool.tile([128, H], fp32, tag="hf")
                nc.vector.tensor_copy(hf[:], ps[:, 0:H])
                nc.vector.tensor_tensor(hf[:], hf[:], ps[:, H:2 * H],
                                        op=mybir.AluOpType.add)
                if hilo:
                    nc.vector.tensor_copy(h_shard[:, m, 0:H], hf[:])
                    nc.vector.tensor_tensor(h_shard[:, m, H:2 * H], hf[:],
                                            h_shard[:, m, 0:H],
                                            op=mybir.AluOpType.subtract)
                else:
                    nc.vector.tensor_copy(h_shard[:, m, :], hf[:])

            def gather(h_shard):
                """AllGather h_shard [128, M_TILES, HW] -> t_hfull."""
                sh_dram = dpool.tile([PER_CORE, HW], h_dt, tag="sh_dram")
                g_dram = dpool.tile([NP, HW], h_dt, tag="g_dram",
                                    addr_space="Shared")
                nc.sync.dma_start(
                    sh_dram[:].rearrange("(m p) f -> p m f", p=128), h_shard[:])
                if single_core:
                    for c in range(N_CORES):
                        nc.sync.dma_start(
                            t_hfull[:, c * M_TILES:(c + 1) * M_TILES, :],
                            sh_dram[:].rearrange("(m p) f -> p m f", p=128))
                else:
                    nc.gpsimd.collective_compute(
                        "AllGather", mybir.AluOpType.bypass,
                        replica_groups=[list(range(N_CORES))],
                        ins=[sh_dram.opt()], outs=[g_dram.opt()],
                    )
                    nc.sync.dma_start(
                        t_hfull[:], g_dram[:].rearrange("(k p) f -> p k f", p=128))

            def aggregate(m, t_bias):
                """t_agg[:, m, :] = A_hat-slice @ h + bias, plus row sum-squares."""
                if hilo:
                    acol = apool.tile([128, K_TILES, 128], a_dt, tag="acol")
                    nc.sync.dma_start(acol[:], a_pre[m])
                    a_of = lambda k: acol[:, k, :]
                else:
                    a_of = lambda k: t_acache[:, m, k, :]
                ps = pagg.tile([128, HW], fp32, tag="ps_agg")
                for k in range(K_TILES):
                    nc.tensor.matmul(ps[:], a_of(k), t_hfull[:, k, :],
                                     start=(k == 0), stop=(k == K_TILES - 1))
                nc.vector.tensor_tensor(t_agg[:, m, :], ps[:, 0:H], t_bias[:],
                                        op=mybir.AluOpType.add)
                if hilo:
                    nc.vector.tensor_tensor(t_agg[:, m, :], t_agg[:, m, :],
                                            ps[:, H:2 * H],
                                            op=mybir.AluOpType.add)
                nc.scalar.activation(t_sq_scratch[:], t_agg[:, m, :],
                                     mybir.ActivationFunctionType.Square,
                                     accum_out=t_ssq[:, m:m + 1])

            t_n2 = wpool.tile([128, M_TILES], fp32)
            t_inv2 = wpool.tile([128, M_TILES], fp32)

            for _rep in range(repeats):
              # =============== Layer 1 ===============
              # h1 shard for own nodes: lhsT = X^T[:, own] chunks (K = F)
              h1_shard = spool.tile([128, M_TILES, HW], h_dt, tag="h_shard",
                                    name=f"h1_shard_{_rep}")
              for m in range(M_TILES):
                compute_h_shard(
                    m,
                    lambda k, m=m: t_xt_hi[:, k, m * 128:(m + 1) * 128],
                    lambda k, m=m: t_xt_lo[:, k, m * 128:(m + 1) * 128],
                    lambda k: t_w1[:, k, :],
                    2,
                    h1_shard,
                )
              gather(h1_shard)

              # ====== Layer 1 aggregation fused with norm/relu/h2 per m-tile
              # so the postprocessing overlaps the next m-tile's agg matmuls.
              h2_shard = spool.tile([128, M_TILES, HW], h_dt, tag="h_shard",
                                    name=f"h2_shard_{_rep}")
              for m in range(M_TILES):
                aggregate(m, t_b1)
                msl = slice(m, m + 1)
                nc.scalar.sqrt(t_nrm[:, msl], t_ssq[:, msl])
                nc.vector.tensor_scalar_max(t_nrm[:, msl], t_nrm[:, msl], 1e-12)
                nc.vector.reciprocal(t_inv[:, msl], t_nrm[:, msl])
                # x1 = relu(agg1 / max(||agg1||, 1e-12))
                nc.scalar.activation(t_x1[:, m, :], t_agg[:, m, :],
                                     mybir.ActivationFunctionType.Relu,
                                     scale=t_inv[:, msl])
                # transpose x1 tile, split to bf16 hi/lo
                pst = ptr.tile([128, 128], fp32, tag="pst")
                nc.tensor.transpose(pst[:], t_x1[:, m, :], t_id[:])
                nc.vector.tensor_copy(t_x1t_hi[:, m, :], pst[:])
                nc.vector.tensor_tensor(t_x1t_lo[:, m, :], pst[:],
                                        t_x1t_hi[:, m, :],
                                        op=mybir.AluOpType.subtract)
                # h2 shard for this m-tile
                compute_h_shard(
                    m,
                    lambda k, m=m: t_x1t_hi[:, m, :],
                    lambda k, m=m: t_x1t_lo[:, m, :],
                    lambda k: t_w2[:],
                    1,
                    h2_shard,
                )
              gather(h2_shard)

              # ====== Layer 2 aggregation fused with the final double l2norm.
              for m in range(M_TILES):
                aggregate(m, t_b2)
                msl = slice(m, m + 1)
                # out = l2norm(l2norm(agg2)): y = x*inv1; ||y|| = ||x||*inv1
                nc.scalar.sqrt(t_nrm[:, msl], t_ssq[:, msl])
                nc.vector.tensor_scalar_max(t_nrm[:, msl], t_nrm[:, msl], 1e-12)
                nc.vector.reciprocal(t_inv[:, msl], t_nrm[:, msl])
                nc.vector.tensor_tensor(t_n2[:, msl], t_nrm[:, msl],
                                        t_inv[:, msl], op=mybir.AluOpType.mult)
                nc.vector.tensor_scalar_max(t_n2[:, msl], t_n2[:, msl], 1e-12)
                nc.vector.reciprocal(t_inv2[:, msl], t_n2[:, msl])
                nc.vector.tensor_tensor(t_inv[:, msl], t_inv[:, msl],
                                        t_inv2[:, msl], op=mybir.AluOpType.mult)
                nc.scalar.activation(t_outf[:, m, :], t_agg[:, m, :],
                                     mybir.ActivationFunctionType.Copy,
                                     scale=t_inv[:, msl])
                nc.sync.dma_start(
                    out[:].rearrange("(mm p) f -> p mm f", p=128)[:, m, :],
                    t_outf[:, m, :])

    if compile:
        nc.compile()
    return nc


def _prep_inputs(X, A, W1, b1, W2, b2, agg_mode):
    """Host-side sharding/layout prep. Returns in_maps for the 8 cores."""
    f32 = np.float32
    a_np_dt = BF16 if agg_mode == "bf16x2" else ml_dtypes.float8_e4m3

    # --- A_hat^T = binarized A with unit diagonal, padded, tiled ---
    Ab = np.zeros((NP, NP), dtype=a_np_dt)
    Ab[:N, :N] = (A != 0)
    idx = np.arange(N)
    Ab[idx, idx] = 1.0
    # [k, p, cm, n] -> [cm, p, k, n]
    T = Ab.reshape(K_TILES, 128, K_TILES, 128).transpose(2, 1, 0, 3)

    # --- X^T hi/lo, padded ---
    Xp = np.zeros((NP, F), dtype=f32)
    Xp[:N] = np.asarray(X, dtype=f32)
    XT = np.ascontiguousarray(Xp.T)                  # [256, NP]
    XT_hi, XT_lo = _split_hilo(XT)

    # --- W1 [F, H] -> [128, 2, 2H] (p, k2, hi|lo) ---
    W1r = np.asarray(W1, dtype=f32).reshape(2, 128, H)
    W1_hi, W1_lo = _split_hilo(W1r)
    w1_cat = np.concatenate([W1_hi, W1_lo], axis=2)   # [2, 128, 2H]
    w1_host = np.ascontiguousarray(w1_cat.transpose(1, 0, 2))  # [128, 2, 2H]

    # --- W2 [H, H] -> [128, 2H] hi|lo ---
    W2_hi, W2_lo = _split_hilo(np.asarray(W2, dtype=f32))
    w2_host = np.ascontiguousarray(np.concatenate([W2_hi, W2_lo], axis=1))

    b1_host = np.ascontiguousarray(
        np.broadcast_to(np.asarray(b1, dtype=f32), (128, H)))
    b2_host = np.ascontiguousarray(
        np.broadcast_to(np.asarray(b2, dtype=f32), (128, H)))
    ident = np.eye(128, dtype=f32)

    in_maps = []
    for c in range(N_CORES):
        cols = slice(c * PER_CORE, (c + 1) * PER_CORE)
        a_pre_c = np.ascontiguousarray(T[c * M_TILES:(c + 1) * M_TILES])
        xt_hi_c = np.ascontiguousarray(
            XT_hi[:, cols].reshape(2, 128, PER_CORE).transpose(1, 0, 2))
        xt_lo_c = np.ascontiguousarray(
            XT_lo[:, cols].reshape(2, 128, PER_CORE).transpose(1, 0, 2))
        in_maps.append({
            "a_pre": a_pre_c,
            "xt_hi": xt_hi_c,
            "xt_lo": xt_lo_c,
            "w1": w1_host,
            "w2": w2_host,
            "b1": b1_host,
            "b2": b2_host,
            "ident": ident,
        })
    return in_maps


def _get_nc(agg_mode=None):
    agg_mode = agg_mode or AGG_MODE
    key = f"nc_{agg_mode}"
    if key not in _CACHE:
        _CACHE[key] = _build_nc(agg_mode)
    return _CACHE[key]


def kernel(X, A, W1, b1, W2, b2, _trace=False, _trace_kwargs=None):
    nc = _get_nc()
    in_maps = _prep_inputs(X, A, W1, b1, W2, b2, AGG_MODE)
    kw = {}
    if _trace:
        kw.update(trace=True, **(_trace_kwargs or {}))
    res = bass_utils.run_bass_kernel_spmd(
        nc, in_maps, core_ids=list(range(N_CORES)), **kw)
    _CACHE["last_result"] = res
    out = np.concatenate([res.results[c]["out"] for c in range(N_CORES)],
                         axis=0)[:N]
    return np.ascontiguousarray(out.astype(np.float32))



# revision 3
# speedup vs baseline: 1.8458x; 1.8458x over previous
"""Trainium2 Bass kernel for nn_Net_43052752175597 (2-layer GraphSAGE, aggr=add).

reference:
    A_hat = (A != 0).T with unit diagonal          # [N, N], binary
    h1   = X @ W1;  agg1 = A_hat @ h1 + b1;  x1 = relu(l2norm(agg1))
    h2   = x1 @ W2; agg2 = A_hat @ h2 + b2;  out = l2norm(l2norm(agg2))

Sharding: row-shard A_hat (output nodes) across 8 cores. Each core owns a
contiguous block of 1280 padded nodes (N padded 10000 -> 10240). Per layer the
core computes h for its own nodes in fp16, AllGathers the transformed
features in two chunks (so the collectives and the post-collective SBUF
loads overlap the aggregation matmuls), then aggregates its A_hat row-slice
(fp8 tiles, SBUF-resident across both layers) against the gathered features.

The gathered-feature SBUF buffer is double-buffered across layers so layer-2
chunks can land while layer-1 aggregation still reads the layer-1 features.
The global source-node (k) order is permuted chunk-major/rank-major to match
the chunked AllGather output layout; the host-side A_hat^T tiling applies the
same permutation, so on-device indexing stays trivial.

Precision: A is exactly 0/1 so fp8 A tiles are exact. h flows as fp16
(2^-11 rounding) with f32 PSUM accumulation -> ~1e-4 final rel err.
"""

import sys

sys.path.insert(0, "/opt/trn_rl_repo")

import numpy as np
import ml_dtypes

import concourse.bass as bass
import concourse.tile as tile
from concourse import bacc, mybir
from concourse import bass_utils

N = 10000
NP = 10240          # padded node count
F = 256             # input feature dim
H = 128             # hidden dim
N_CORES = 8
PER_CORE = NP // N_CORES        # 1280 nodes per core
M_TILES = PER_CORE // 128       # 10
K_TILES = NP // 128             # 80
N_CHUNKS = 2                    # AllGather chunks per layer
MC = M_TILES // N_CHUNKS        # own m-tiles per chunk (5)
KC = K_TILES // N_CHUNKS        # gathered k-tiles per chunk (40)

AGG_MODE = "fp16"

_CACHE = {}


def _build_nc(agg_mode=AGG_MODE, single_core=False, compile=True, repeats=1):
    """Build + compile the 8-core SPMD Bass kernel. Returns the Bacc object.

    single_core=True builds a 1-core variant with the collectives replaced by
    equivalent-byte local DMAs — only for TimelineSim cost-model profiling.
    repeats>1 runs the whole 2-layer body N times (benchmarking only).
    """
    fp32 = mybir.dt.float32
    fp16 = mybir.dt.float16
    fp8 = mybir.dt.float8e4

    nc = bacc.Bacc(
        "TRN2",
        target_bir_lowering=False,
        debug=False,
        enable_asserts=True,
        num_devices=1 if single_core else N_CORES,
    )

    # Per-core external inputs
    a_pre = nc.dram_tensor("a_pre", [M_TILES, 128, K_TILES, 128], fp8,
                           kind="ExternalInput").ap()
    xt = nc.dram_tensor("xt", [128, 2, PER_CORE], fp16,
                        kind="ExternalInput").ap()
    w1 = nc.dram_tensor("w1", [128, 2, H], fp16, kind="ExternalInput").ap()
    w2 = nc.dram_tensor("w2", [128, H], fp16, kind="ExternalInput").ap()
    b1 = nc.dram_tensor("b1", [128, H], fp32, kind="ExternalInput").ap()
    b2 = nc.dram_tensor("b2", [128, H], fp32, kind="ExternalInput").ap()
    ident = nc.dram_tensor("ident", [128, 128], fp32, kind="ExternalInput").ap()
    out = nc.dram_tensor("out", [PER_CORE, H], fp32, kind="ExternalOutput").ap()

    with tile.TileContext(nc) as tc:
        with tc.tile_pool(name="const", bufs=1) as cpool, \
             tc.tile_pool(name="hfull", bufs=2) as hpool, \
             tc.tile_pool(name="acol", bufs=1) as apool, \
             tc.tile_pool(name="work", bufs=1) as wpool, \
             tc.tile_pool(name="shard", bufs=2) as spool, \
             tc.tile_pool(name="psum_agg", bufs=2, space="PSUM") as pagg, \
             tc.tile_pool(name="psum_aux", bufs=2, space="PSUM") as paux, \
             tc.tile_pool(name="psum_tr", bufs=2, space="PSUM") as ptr, \
             tc.tile_pool(name="dram", bufs=2, space="DRAM") as dpool:

            # ---- constants into SBUF (gather-critical path first) ----
            t_xt = cpool.tile([128, 2, PER_CORE], fp16)
            t_w1 = cpool.tile([128, 2, H], fp16)
            t_w2 = cpool.tile([128, H], fp16)
            t_b1 = cpool.tile([128, H], fp32)
            t_b2 = cpool.tile([128, H], fp32)
            t_id = cpool.tile([128, 128], fp32)
            nc.sync.dma_start(t_xt[:], xt[:])
            nc.sync.dma_start(t_w1[:], w1[:])
            nc.sync.dma_start(t_w2[:], w2[:])
            nc.sync.dma_start(t_b1[:], b1[:])
            nc.sync.dma_start(t_b2[:], b2[:])
            nc.sync.dma_start(t_id[:], ident[:])

            # Whole per-core A slice stays resident in SBUF, loaded once
            # (m-major so early m-tiles can start ASAP) and reused by both
            # layers. SWDGE so these bulk loads don't queue ahead of the
            # latency-critical HWDGE gather/feature DMAs.
            t_acache = apool.tile([128, M_TILES, K_TILES, 128], fp8)
            for m in range(M_TILES):
                nc.gpsimd.dma_start(t_acache[:, m, :, :], a_pre[m])

            # per-layer working tiles
            t_agg = wpool.tile([128, M_TILES, H], fp32, tag="agg")
            t_x1 = wpool.tile([128, M_TILES, H], fp32)
            t_x1t = wpool.tile([128, M_TILES, H], fp16)
            t_ssq = wpool.tile([128, M_TILES], fp32, tag="ssq")
            t_nrm = wpool.tile([128, M_TILES], fp32, tag="nrm")
            t_inv = wpool.tile([128, M_TILES], fp32, tag="inv")
            t_sq_scratch = wpool.tile([128, H], fp32)
            t_outf = wpool.tile([128, M_TILES, H], fp32)

            def gather_chunk(h_shard, ck, t_hfull, tag):
                """AllGather own m-tiles [ck*MC, (ck+1)*MC) of h_shard into
                t_hfull slots [ck*KC, (ck+1)*KC) (rank-major layout)."""
                msl = slice(ck * MC, (ck + 1) * MC)
                sh_dram = dpool.tile([MC * 128, H], fp16, tag="sh_dram",
                                     name=f"sh_{tag}")
                g_dram = dpool.tile([N_CORES * MC * 128, H], fp16,
                                    tag="g_dram", name=f"g_{tag}",
                                    addr_space="Shared")
                nc.sync.dma_start(
                    sh_dram[:].rearrange("(m p) f -> p m f", p=128),
                    h_shard[:, msl, :])
                if single_core:
                    for r in range(N_CORES):
                        ks = ck * KC + r * MC
                        nc.sync.dma_start(
                            t_hfull[:, ks:ks + MC, :],
                            sh_dram[:].rearrange("(m p) f -> p m f", p=128))
                else:
                    nc.gpsimd.collective_compute(
                        "AllGather", mybir.AluOpType.bypass,
                        replica_groups=[list(range(N_CORES))],
                        ins=[sh_dram.opt()], outs=[g_dram.opt()],
                    )
                    # split the SBUF load so aggregation can start on the
                    # first ranks' k-tiles while the rest still loads
                    kh = KC // 2
                    gv = g_dram[:].rearrange("(k p) f -> p k f", p=128)
                    nc.sync.dma_start(
                        t_hfull[:, ck * KC:ck * KC + kh, :], gv[:, 0:kh, :])
                    nc.sync.dma_start(
                        t_hfull[:, ck * KC + kh:(ck + 1) * KC, :],
                        gv[:, kh:KC, :])

            def aggregate(m, t_bias, t_hfull):
                """t_agg[:, m, :] = A_hat-slice @ h + bias, plus row sum-sq."""
                ps = pagg.tile([128, H], fp32, tag="ps_agg")
                for k in range(K_TILES):
                    nc.tensor.matmul(ps[:], t_acache[:, m, k, :],
                                     t_hfull[:, k, :],
                                     start=(k == 0), stop=(k == K_TILES - 1))
                nc.vector.tensor_tensor(t_agg[:, m, :], ps[:, 0:H], t_bias[:],
                                        op=mybir.AluOpType.add)
                nc.scalar.activation(t_sq_scratch[:], t_agg[:, m, :],
                                     mybir.ActivationFunctionType.Square,
                                     accum_out=t_ssq[:, m:m + 1])

            for _rep in range(repeats):
              t_h1full = hpool.tile([128, K_TILES, H], fp16, tag="hfull",
                                    name=f"h1full_{_rep}")
              t_h2full = hpool.tile([128, K_TILES, H], fp16, tag="hfull",
                                    name=f"h2full_{_rep}")

              # =============== Layer 1: h1 = fp16(X @ W1) for own nodes ====
              h1_shard = spool.tile([128, M_TILES, H], fp16, tag="h_shard",
                                    name=f"h1_shard_{_rep}")
              for ck in range(N_CHUNKS):
                for m in range(ck * MC, (ck + 1) * MC):
                    ps = paux.tile([128, H], fp32, tag="ps_h")
                    for k in range(2):
                        nc.tensor.matmul(ps[:],
                                         t_xt[:, k, m * 128:(m + 1) * 128],
                                         t_w1[:, k, :],
                                         start=(k == 0), stop=(k == 1))
                    nc.vector.tensor_copy(h1_shard[:, m, :], ps[:])
                gather_chunk(h1_shard, ck, t_h1full, f"h1c{ck}_{_rep}")

              # ====== Layer 1 aggregation fused with norm/relu/h2 per m-tile
              # so the postprocessing overlaps the next m-tile's agg matmuls.
              h2_shard = spool.tile([128, M_TILES, H], fp16, tag="h_shard",
                                    name=f"h2_shard_{_rep}")
              for m in range(M_TILES):
                aggregate(m, t_b1, t_h1full)
                msl = slice(m, m + 1)
                nc.scalar.sqrt(t_nrm[:, msl], t_ssq[:, msl])
                nc.vector.tensor_scalar_max(t_nrm[:, msl], t_nrm[:, msl], 1e-12)
                nc.vector.reciprocal(t_inv[:, msl], t_nrm[:, msl])
                # x1 = relu(agg1 / max(||agg1||, 1e-12))
                nc.scalar.activation(t_x1[:, m, :], t_agg[:, m, :],
                                     mybir.ActivationFunctionType.Relu,
                                     scale=t_inv[:, msl])
                # transpose x1 tile -> fp16 stationary for h2
                pst = ptr.tile([128, 128], fp32, tag="pst")
                nc.tensor.transpose(pst[:], t_x1[:, m, :], t_id[:])
                nc.vector.tensor_copy(t_x1t[:, m, :], pst[:])
                ps2 = paux.tile([128, H], fp32, tag="ps_h")
                nc.tensor.matmul(ps2[:], t_x1t[:, m, :], t_w2[:],
                                 start=True, stop=True)
                nc.vector.tensor_copy(h2_shard[:, m, :], ps2[:])
                if m == MC - 1:
                    gather_chunk(h2_shard, 0, t_h2full, f"h2c0_{_rep}")
                elif m == M_TILES - 1:
                    gather_chunk(h2_shard, 1, t_h2full, f"h2c1_{_rep}")

              # ====== Layer 2 aggregation fused with the final l2norm.
              # l2norm(l2norm(x)) == l2norm(x) up to f32 rounding whenever
              # ||x|| > eps (always: agg2 includes the b2 offset), since the
              # inner normalize yields a unit-norm vector.
              for m in range(M_TILES):
                aggregate(m, t_b2, t_h2full)
                msl = slice(m, m + 1)
                nc.scalar.sqrt(t_nrm[:, msl], t_ssq[:, msl])
                nc.vector.tensor_scalar_max(t_nrm[:, msl], t_nrm[:, msl], 1e-12)
                nc.vector.reciprocal(t_inv[:, msl], t_nrm[:, msl])
                nc.scalar.activation(t_outf[:, m, :], t_agg[:, m, :],
                                     mybir.ActivationFunctionType.Copy,
                                     scale=t_inv[:, msl])
                nc.sync.dma_start(
                    out[:].rearrange("(mm p) f -> p mm f", p=128)[:, m, :],
                    t_outf[:, m, :])

    if compile:
        nc.compile()
    return nc


def _k_perm():
    """New k-tile order: chunk-major, rank-major, tile-minor.
    perm[new_k] = old_k where old_k = rank*M_TILES + chunk*MC + t."""
    perm = np.empty(K_TILES, dtype=np.int64)
    for nk in range(K_TILES):
        ck, w = divmod(nk, KC)
        r, t = divmod(w, MC)
        perm[nk] = r * M_TILES + ck * MC + t
    return perm


def _prep_inputs(X, A, W1, b1, W2, b2, agg_mode=AGG_MODE):
    """Host-side sharding/layout prep. Returns in_maps for the 8 cores."""
    f32 = np.float32
    fp16 = np.float16
    a_np_dt = ml_dtypes.float8_e4m3

    # --- A_hat^T = binarized A with unit diagonal, padded, tiled ---
    Ab = np.zeros((NP, NP), dtype=a_np_dt)
    Ab[:N, :N] = (A != 0)
    idx = np.arange(N)
    Ab[idx, idx] = 1.0
    # [k, p, cm, n] -> [cm, p, k, n], then permute k to the chunked layout
    T = Ab.reshape(K_TILES, 128, K_TILES, 128).transpose(2, 1, 0, 3)
    T = T[:, :, _k_perm(), :]

    # --- X^T fp16, padded ---
    Xp = np.zeros((NP, F), dtype=f32)
    Xp[:N] = np.asarray(X, dtype=f32)
    XT = np.ascontiguousarray(Xp.T).astype(fp16)     # [256, NP]

    w1_host = np.ascontiguousarray(
        np.asarray(W1, dtype=f32).reshape(2, 128, H).transpose(1, 0, 2)
    ).astype(fp16)                                   # [128, 2, H]
    w2_host = np.asarray(W2, dtype=f32).astype(fp16)  # [128, H]

    b1_host = np.ascontiguousarray(
        np.broadcast_to(np.asarray(b1, dtype=f32), (128, H)))
    b2_host = np.ascontiguousarray(
        np.broadcast_to(np.asarray(b2, dtype=f32), (128, H)))
    ident = np.eye(128, dtype=f32)

    in_maps = []
    for c in range(N_CORES):
        cols = slice(c * PER_CORE, (c + 1) * PER_CORE)
        a_pre_c = np.ascontiguousarray(T[c * M_TILES:(c + 1) * M_TILES])
        xt_c = np.ascontiguousarray(
            XT[:, cols].reshape(2, 128, PER_CORE).transpose(1, 0, 2))
        in_maps.append({
            "a_pre": a_pre_c,
            "xt": xt_c,
            "w1": w1_host,
            "w2": w2_host,
            "b1": b1_host,
            "b2": b2_host,
            "ident": ident,
        })
    return in_maps


def _get_nc(agg_mode=None):
    key = f"nc_{agg_mode or AGG_MODE}"
    if key not in _CACHE:
        _CACHE[key] = _build_nc(agg_mode or AGG_MODE)
    return _CACHE[key]


def kernel(X, A, W1, b1, W2, b2, _trace=False, _trace_kwargs=None):
    nc = _get_nc()
    in_maps = _prep_inputs(X, A, W1, b1, W2, b2, AGG_MODE)
    kw = {}
    if _trace:
        kw.update(trace=True, **(_trace_kwargs or {}))
    res = bass_utils.run_bass_kernel_spmd(
        nc, in_maps, core_ids=list(range(N_CORES)), **kw)
    _CACHE["last_result"] = res
    out = np.concatenate([res.results[c]["out"] for c in range(N_CORES)],
                         axis=0)[:N]
    return np.ascontiguousarray(out.astype(np.float32))
